# revision 4
# baseline (speedup 1.0000x reference)
"""Trainium2 Bass kernel for AllPassMORRCirculantLinear.

Math (reference, per batch row b):
  xb = x.reshape(bs, q, k); xb = xb*xb
  phi[b,p,q,t] = sum_s xb[b,q,s] * |w|[p,q,(t-s) mod k]   (circular conv, k=8)
  t(phi) = (a^2 + r^2 - 2 a r cos phi) / (1 + (ar)^2 - 2 a r cos phi)
  out[b, p*k+t] = sum_q scale[q] * t(phi[b,p,q,t])

Using t = 1 - K*u, u = 1/(B - 2*rho*cos phi), sum_q scale[q] == 0:
  out = sum_q s'_q * u_q,  s'_q = -K*scale[q]
Half-angle form (pole-robust):
  B - 2 rho cos phi = (1-rho)^2 + 4 rho sin^2(phi/2)
  sin^2(phi/2) = sin^2(pi * frac(psi)),  psi = phi/(2 pi)   (sign-free!)

Per-core pipeline (data-parallel over batch, 128 rows/core):
  PE  : psi = phi/(2pi) via fp16 hi/lo matmuls (1/2pi folded into weights,
        22-bit effective mantissa) -> PSUM [128,1024] per q
  DVE : one 8-stage custom op FRAC_SINSQ: magic-number frac (exact) +
        minimax cubic s = r*(a + b r^2), v = s^2 ~ sin^2(pi r)
        (relative error weighted by pole sensitivity: 1.2e-2 max)
  ACT : u'_q = Reciprocal(scale_q * v + bias_q)  [raw InstActivation;
        scale_q = 4rho/s'_q, bias_q = (1-rho)^2/s'_q as immediates]
  Pool: pair-sums u'_{2j}+u'_{2j+1} (fp16)
  PE  : out += I @ pair  (fp16 identity-stationary accumulating matmuls
        into a dedicated PSUM bank pair; start at pair 0, stop at 63)
All ACT funcs (Square/Copy/Reciprocal/Identity) live in one table set
(reciprocal_and_small) -> single ACT_TABLE_LOAD.
"""

import sys

for _p in ("/opt/trn_rl_repo",):
    if _p not in sys.path:
        sys.path.insert(0, _p)

import numpy as np
from contextlib import ExitStack

MRR_A = 0.8682
MRR_R = 0.8602
RHO = MRR_A * MRR_R
MCONST = (1.0 - RHO) ** 2                      # (1-rho)^2
KCONST = (1.0 - MRR_A * MRR_A) * (1.0 - MRR_R * MRR_R)

BS, IN_CH, OUT_CH, KB = 1024, 1024, 1024, 8
Q = IN_CH // KB    # 128
P = OUT_CH // KB   # 128
NCORES = 8
BSC = BS // NCORES  # 128 batch rows per core
NCHUNK = 16        # q chunks of 8

MAGIC = 12582912.0  # 1.5*2^23: x + MAGIC - MAGIC == round(x) in fp32 RNE
INV2PI = 1.0 / (2.0 * np.pi)
POLY_A = 3.111744676   # minimax (pole-sensitivity weighted): sin(pi r) ~ r(A + B r^2)
POLY_B = -4.497768741

_CACHE = {}


def _sinsq_ref(in0, in1, s0, s1, imm2):
    f = np.float32
    x = in0.astype(f)
    t = (x + f(s0)).astype(f)
    k = (t - f(s0)).astype(f)
    r = (x - k).astype(f)
    r2 = (r * r).astype(f)
    g = ((r2 * f(imm2)).astype(f) + f(s1)).astype(f)
    st = (r * g).astype(f)
    return (st * st).astype(f)


def _register_frac_sinsq():
    from concourse import dve_ops
    from concourse.dve_spec import Spec, Src0, C0, C1, C2, lower
    from concourse.dve_uop import DveOpSpec

    name = "FRAC_SINSQ_ANT"
    if name in dve_ops._SUB_OPCODE_FOR_NAME:
        return next(op for op in dve_ops.OPS if op.name == name)
    t = Src0 + C0
    k = t - C0
    r = Src0 - k
    r2 = r * r
    g = r2 * C2 + C1
    st = r * g
    spec = Spec(body=st * st, reference=_sinsq_ref)
    row = max(dve_ops._SUB_OPCODE_FOR_NAME.values()) + 1
    assert row < 0x20
    dve_ops._SUB_OPCODE_FOR_NAME[name] = row
    shas = {}
    for ver in ("v3", "v4"):
        c = DveOpSpec(name=name, opcode=row, uops=lower(spec, ver=ver), rd1_en=False)
        shas[ver] = c.sha(ver)
    op = dve_ops.DveOp(name, spec, subdim=False, uops_sha=shas)
    dve_ops.OPS.append(op)
    dve_ops.CUSTOM_DVE_SPECS[name] = spec
    return op


def _raw_act(nc, out, in_, func, bias=0.0, scale=1.0):
    """Emit InstActivation directly (the bass wrapper blocks Reciprocal;
    measured accuracy here is ~1e-5 relative, ample for this kernel)."""
    from concourse import mybir
    se = nc.scalar
    inputs = [se.lower_ap(in_)]
    for arg in (bias, scale, 0.0):
        inputs.append(mybir.ImmediateValue(dtype=mybir.dt.float32, value=float(arg)))
    return se.add_instruction(
        mybir.InstActivation(
            name=nc.get_next_instruction_name(),
            func=func,
            ins=inputs,
            outs=[se.lower_ap(out)],
        )
    )


def _build_nc(scales, biases, niter=1):
    from concourse import bacc, mybir
    import concourse.tile as tile
    from concourse import masks

    sinsq_op = _register_frac_sinsq()

    nc = bacc.Bacc("TRN2", debug=False)
    f32 = mybir.dt.float32
    f16 = mybir.dt.float16
    AF = mybir.ActivationFunctionType

    x_d = nc.dram_tensor("x", [BSC, IN_CH], f32, kind="ExternalInput")
    wc1_d = nc.dram_tensor("wc1", [KB, Q, OUT_CH], f16, kind="ExternalInput")
    wc2_d = nc.dram_tensor("wc2", [2 * KB, Q, OUT_CH], f16, kind="ExternalInput")
    out_d = nc.dram_tensor("out", [BSC, OUT_CH], f32, kind="ExternalOutput")

    with tile.TileContext(nc) as tc:
        with ExitStack() as ctx:
            singles = ctx.enter_context(tc.tile_pool(name="singles", bufs=1))
            psum_out = ctx.enter_context(
                tc.tile_pool(name="psout", bufs=1, space="PSUM"))
            psum = ctx.enter_context(tc.tile_pool(name="psum", bufs=3, space="PSUM"))
            wqp = ctx.enter_context(tc.tile_pool(name="wqp", bufs=2))
            vpool = ctx.enter_context(tc.tile_pool(name="vpool", bufs=4))
            upool = ctx.enter_context(tc.tile_pool(name="upool", bufs=4))
            ppool = ctx.enter_context(tc.tile_pool(name="ppool", bufs=8))

            ident = singles.tile([128, 128], f16)
            masks.make_identity(nc, ident[:])

            x_sb = singles.tile([128, IN_CH], f32)
            nc.sync.dma_start(x_sb[:], x_d.ap())
            # input intensity modulation: x <- x^2 (in place)
            nc.scalar.activation(x_sb[:], x_sb[:], AF.Square)

            # interleaved hi/lo staging tile: [part, q, s-slot(16)]
            # slots 0..7 = fp16(xq), slots 8..15 = residual
            xhl = singles.tile([128, Q, 16], f16)
            xq3 = x_sb[:].rearrange("p (q s) -> p q s", s=KB)
            nc.scalar.copy(xhl[:, :, 0:KB], xq3)
            nc.vector.tensor_sub(xhl[:, :, KB:2 * KB], xq3, xhl[:, :, 0:KB])

            # transpose to xst[g][s(16), j(8), b(128)] per 8-q group
            xsts = []
            if True:
                for g in range(NCHUNK):
                    tp = psum.tile([16, KB * 128], f16, tag="ps")
                    for j in range(KB):
                        nc.tensor.transpose(
                            tp[:, j * 128:(j + 1) * 128],
                            xhl[:, g * KB + j, :], ident[:])
                    xst = singles.tile([16, KB, 128], f16, tag=f"xst{g}")
                    nc.scalar.copy(
                        xst[:].rearrange("s j b -> s (j b)"), tp[:])
                    xsts.append(xst)

            out_ps = psum_out.tile([128, OUT_CH], f32)

            def run_iter():
                pend = []  # pair tiles awaiting PE accumulation
                nacc = [0]

                def emit_acc(tiles, last):
                    for up in tiles:
                        for h in range(2):
                            nc.tensor.matmul(
                                out_ps[:, h * 512:(h + 1) * 512],
                                ident[:], up[:, h * 512:(h + 1) * 512],
                                start=(nacc[0] == 0),
                                stop=(last and up is tiles[-1] and h == 1),
                                skip_group_check=True, tile_position=(0, 0))
                        nacc[0] += 1

                for c in range(NCHUNK):
                    q8 = c * KB
                    wq1 = wqp.tile([KB, KB, OUT_CH], f16, tag="wq1")
                    nc.sync.dma_start(wq1[:], wc1_d.ap()[:, q8:q8 + KB, :])
                    wq2 = wqp.tile([2 * KB, KB, OUT_CH], f16, tag="wq2")
                    nc.sync.dma_start(wq2[:], wc2_d.ap()[:, q8:q8 + KB, :])

                    us = []
                    for j in range(KB):
                        q = q8 + j
                        psi = psum.tile([128, OUT_CH], f32, tag="ps")
                        for h in range(2):
                            dst = psi[:, h * 512:(h + 1) * 512]
                            nc.tensor.matmul(
                                dst, xsts[c][0:KB, j, :],
                                wq1[:, j, h * 512:(h + 1) * 512],
                                start=True, stop=False,
                                skip_group_check=True, tile_position=(0, 0))
                            nc.tensor.matmul(
                                dst, xsts[c][:, j, :],
                                wq2[:, j, h * 512:(h + 1) * 512],
                                start=False, stop=True,
                                skip_group_check=True, tile_position=(0, 0))
                        v = vpool.tile([128, OUT_CH], f16, tag="v")
                        nc.vector._custom_dve(
                            sinsq_op, out=v[:], in0=psi[:],
                            s0=MAGIC, s1=POLY_A, imm2=POLY_B)
                        u = upool.tile([128, OUT_CH], f16, tag="u")
                        _raw_act(nc, u[:], v[:], AF.Reciprocal,
                                 bias=biases[q], scale=scales[q])
                        us.append(u)
                    pairs = []
                    for jj in range(KB // 2):
                        up = ppool.tile([128, OUT_CH], f16, tag="up")
                        nc.gpsimd.tensor_add(up[:], us[2 * jj][:], us[2 * jj + 1][:])
                        pairs.append(up)
                    # lag PE accumulation by one chunk so PE never waits
                    if pend:
                        emit_acc(pend, last=False)
                    pend = pairs
                emit_acc(pend, last=True)

            if niter == 1:
                run_iter()
            else:
                with tc.For_i(0, niter, 1):
                    run_iter()

            out_sb = singles.tile([128, OUT_CH], f32)
            nc.scalar.copy(out_sb[:], out_ps[:])
            nc.sync.dma_start(out_d.ap(), out_sb[:])

    nc.compile()
    return nc


def _host_prep(weight, morr_output_scale):
    w = np.abs(np.asarray(weight, dtype=np.float32))   # [P, Q, KB]
    s = morr_output_scale - morr_output_scale.mean()
    half = s[..., :-1, :]                              # [1,1,Q//2,1]
    scale = np.concatenate([half, -half], axis=2)[0, 0, :, 0].astype(np.float32)
    sprime = (-KCONST * scale).astype(np.float64)      # folded -K

    # circulant moving-operand layout: wc[s, q, p*KB+t] = w[p, q, (t-s) % KB]
    # with 1/(2 pi) folded in so the matmul yields psi = phi/(2 pi)
    wc = np.empty((KB, Q, P * KB), np.float64)
    for sh in range(KB):
        rolled = np.roll(w, sh, axis=2)
        wc[sh] = rolled.transpose(1, 0, 2).reshape(Q, P * KB)
    wc = (wc * INV2PI).astype(np.float32)

    wh = wc.astype(np.float16)
    wl = (wc - wh.astype(np.float32)).astype(np.float16)
    wq1 = wh                                           # [KB, Q, P*KB]
    wq2 = np.concatenate([wl, wh], axis=0)             # [2KB, Q, P*KB]

    # ACT affine immediates: d_q = (4 rho * v + (1-rho)^2) / s'_q
    with np.errstate(divide="ignore"):
        act_scale = 4.0 * RHO / sprime
        act_bias = MCONST / sprime
    tiny = np.abs(sprime) < 1e-30
    act_scale[tiny] = 0.0
    act_bias[tiny] = 1e30
    return (np.ascontiguousarray(wq1), np.ascontiguousarray(wq2),
            act_scale.astype(np.float64).tolist(),
            act_bias.astype(np.float64).tolist())


def kernel(x, weight, morr_output_scale, _trace=False):
    from concourse import bass_utils

    wq1, wq2, scales, biases = _host_prep(weight, morr_output_scale)
    key = hash((tuple(scales), tuple(biases)))
    if _CACHE.get("key") != key:
        _CACHE["nc"] = _build_nc(scales, biases)
        _CACHE["key"] = key
    nc = _CACHE["nc"]

    x = np.ascontiguousarray(np.asarray(x, dtype=np.float32))
    in_maps = []
    for c in range(NCORES):
        in_maps.append({
            "x": np.ascontiguousarray(x[c * BSC:(c + 1) * BSC]),
            "wc1": wq1, "wc2": wq2,
        })
    res = bass_utils.run_bass_kernel_spmd(
        nc, in_maps, core_ids=list(range(NCORES)), trace=_trace)
    out = np.concatenate([res.results[c]["out"] for c in range(NCORES)], axis=0)
    if _trace:
        _CACHE["last_results"] = res
    return out


# revision 12
# speedup vs baseline: 1.0501x; 1.0501x over previous
"""Trainium2 Bass kernel for AllPassMORRCirculantLinear.

Math (reference, per batch row b):
  xb = x.reshape(bs, q, k); xb = xb*xb
  phi[b,p,q,t] = sum_s xb[b,q,s] * |w|[p,q,(t-s) mod k]   (circular conv, k=8)
  t(phi) = (a^2 + r^2 - 2 a r cos phi) / (1 + (ar)^2 - 2 a r cos phi)
  out[b, p*k+t] = sum_q scale[q] * t(phi[b,p,q,t])

Using t = 1 - K*u, u = 1/(B - 2*rho*cos phi), sum_q scale[q] == 0:
  out = sum_q s'_q * u_q,  s'_q = -K*scale[q]
Half-angle form (pole-robust):
  B - 2 rho cos phi = (1-rho)^2 + 4 rho sin^2(phi/2)
  sin^2(phi/2) = sin^2(pi * frac(psi)),  psi = phi/(2 pi)   (sign-free!)

Per-core pipeline (data-parallel over batch, 128 rows/core):
  PE  : psi = phi/(2pi) via fp16 hi/lo matmuls (1/2pi folded into weights,
        22-bit effective mantissa) -> PSUM [128,1024] per q
  DVE : one 8-stage custom op FRAC_SINSQ: magic-number frac (exact) +
        minimax cubic s = r*(a + b r^2), v = s^2 ~ sin^2(pi r)
        (relative error weighted by pole sensitivity: 1.2e-2 max)
  ACT : u'_q = Reciprocal(scale_q * v + bias_q)  [raw InstActivation;
        scale_q = 4rho/s'_q, bias_q = (1-rho)^2/s'_q as immediates]
  Pool: pair-sums u'_{2j}+u'_{2j+1} (fp16)
  PE  : out += I @ pair  (fp16 identity-stationary accumulating matmuls
        into a dedicated PSUM bank pair; start at pair 0, stop at 63)
All ACT funcs (Square/Copy/Reciprocal/Identity) live in one table set
(reciprocal_and_small) -> single ACT_TABLE_LOAD.
"""

import sys

for _p in ("/opt/trn_rl_repo",):
    if _p not in sys.path:
        sys.path.insert(0, _p)

import numpy as np
from contextlib import ExitStack

MRR_A = 0.8682
MRR_R = 0.8602
RHO = MRR_A * MRR_R
MCONST = (1.0 - RHO) ** 2                      # (1-rho)^2
KCONST = (1.0 - MRR_A * MRR_A) * (1.0 - MRR_R * MRR_R)

BS, IN_CH, OUT_CH, KB = 1024, 1024, 1024, 8
Q = IN_CH // KB    # 128
P = OUT_CH // KB   # 128
NCORES = 8
BSC = BS // NCORES  # 128 batch rows per core
NCHUNK = 16        # q chunks of 8

MAGIC = 12582912.0  # 1.5*2^23: x + MAGIC - MAGIC == round(x) in fp32 RNE
INV2PI = 1.0 / (2.0 * np.pi)
POLY_A = 3.111744676   # minimax (pole-sensitivity weighted): sin(pi r) ~ r(A + B r^2)
POLY_B = -4.497768741

_CACHE = {}


def _sinsq_ref(in0, in1, s0, s1, imm2):
    f = np.float32
    x = in0.astype(f)
    t = (x + f(s0)).astype(f)
    k = (t - f(s0)).astype(f)
    r = (x - k).astype(f)
    r2 = (r * r).astype(f)
    g = ((r2 * f(imm2)).astype(f) + f(s1)).astype(f)
    st = (r * g).astype(f)
    return (st * st).astype(f)


def _register_frac_sinsq():
    from concourse import dve_ops
    from concourse.dve_spec import Spec, Src0, C0, C1, C2, lower
    from concourse.dve_uop import DveOpSpec

    name = "FRAC_SINSQ_ANT"
    if name in dve_ops._SUB_OPCODE_FOR_NAME:
        return next(op for op in dve_ops.OPS if op.name == name)
    t = Src0 + C0
    k = t - C0
    r = Src0 - k
    r2 = r * r
    g = r2 * C2 + C1
    st = r * g
    spec = Spec(body=st * st, reference=_sinsq_ref)
    row = max(dve_ops._SUB_OPCODE_FOR_NAME.values()) + 1
    assert row < 0x20
    dve_ops._SUB_OPCODE_FOR_NAME[name] = row
    shas = {}
    for ver in ("v3", "v4"):
        c = DveOpSpec(name=name, opcode=row, uops=lower(spec, ver=ver), rd1_en=False)
        shas[ver] = c.sha(ver)
    op = dve_ops.DveOp(name, spec, subdim=False, uops_sha=shas)
    dve_ops.OPS.append(op)
    dve_ops.CUSTOM_DVE_SPECS[name] = spec
    return op


def _raw_act(nc, out, in_, func, bias=0.0, scale=1.0):
    """Emit InstActivation directly (the bass wrapper blocks Reciprocal;
    measured accuracy here is ~1e-5 relative, ample for this kernel)."""
    from concourse import mybir
    se = nc.scalar
    inputs = [se.lower_ap(in_)]
    for arg in (bias, scale, 0.0):
        inputs.append(mybir.ImmediateValue(dtype=mybir.dt.float32, value=float(arg)))
    return se.add_instruction(
        mybir.InstActivation(
            name=nc.get_next_instruction_name(),
            func=func,
            ins=inputs,
            outs=[se.lower_ap(out)],
        )
    )


def _build_nc(scales, biases, niter=1):
    from concourse import bacc, mybir
    import concourse.tile as tile
    from concourse import masks

    sinsq_op = _register_frac_sinsq()

    nc = bacc.Bacc("TRN2", debug=False)
    f32 = mybir.dt.float32
    f16 = mybir.dt.float16
    AF = mybir.ActivationFunctionType

    x_d = nc.dram_tensor("x", [BSC, IN_CH], f32, kind="ExternalInput")
    wc2_d = nc.dram_tensor("wc2", [2 * KB, Q, OUT_CH], f16, kind="ExternalInput")
    out_d = nc.dram_tensor("out", [BSC, OUT_CH], f32, kind="ExternalOutput")

    with tile.TileContext(nc) as tc:
        with ExitStack() as ctx:
            singles = ctx.enter_context(tc.tile_pool(name="singles", bufs=1))
            psum_out = ctx.enter_context(
                tc.tile_pool(name="psout", bufs=1, space="PSUM"))
            psum = ctx.enter_context(tc.tile_pool(name="psum", bufs=3, space="PSUM"))
            wqp = ctx.enter_context(tc.tile_pool(name="wqp", bufs=2))
            vpool = ctx.enter_context(tc.tile_pool(name="vpool", bufs=4))
            upool = ctx.enter_context(tc.tile_pool(name="upool", bufs=4))
            ppool = ctx.enter_context(tc.tile_pool(name="ppool", bufs=8))

            ident = singles.tile([128, 128], f16)
            masks.make_identity(nc, ident[:])

            x_sb = singles.tile([128, IN_CH], f32)
            nc.sync.dma_start(x_sb[:], x_d.ap())
            # input intensity modulation: x <- x^2 (in place)
            nc.scalar.activation(x_sb[:], x_sb[:], AF.Square)

            # interleaved hi/lo staging tile: [part, q, s-slot(16)]
            # slots 0..7 = residual (xl), slots 8..15 = fp16(xq) (xh)
            xhl = singles.tile([128, Q, 16], f16)
            xq3 = x_sb[:].rearrange("p (q s) -> p q s", s=KB)
            nc.scalar.copy(xhl[:, :, KB:2 * KB], xq3)
            nc.vector.tensor_sub(xhl[:, :, 0:KB], xq3, xhl[:, :, KB:2 * KB])

            # transpose to xstC[g] = [xl(0-7); xh(8-15)] x [j(8), b(128)];
            # xstA[g] = xh alone at partitions 0-7 (SBUF->SBUF DMA dup)
            xstC, xstA = [], []
            for g in range(NCHUNK):
                tp = psum.tile([16, KB * 128], f16, tag="ps")
                for j in range(KB):
                    nc.tensor.transpose(
                        tp[:, j * 128:(j + 1) * 128],
                        xhl[:, g * KB + j, :], ident[:])
                xc = singles.tile([16, KB, 128], f16, tag=f"xstC{g}")
                nc.scalar.copy(xc[:].rearrange("s j b -> s (j b)"), tp[:])
                xstC.append(xc)
                xa = singles.tile([8, KB, 128], f16, tag=f"xstA{g}")
                nc.gpsimd.dma_start(xa[:], xc[KB:2 * KB])
                xstA.append(xa)

            out_ps = psum_out.tile([128, OUT_CH], f32)

            def run_iter():
                pend = []  # pair tiles awaiting PE accumulation
                nacc = [0]

                def emit_acc(tiles, last):
                    for up in tiles:
                        for h in range(2):
                            nc.tensor.matmul(
                                out_ps[:, h * 512:(h + 1) * 512],
                                ident[:], up[:, h * 512:(h + 1) * 512],
                                start=(nacc[0] == 0),
                                stop=(last and up is tiles[-1] and h == 1),
                                skip_group_check=True, tile_position=(0, 0))
                        nacc[0] += 1

                for c in range(NCHUNK):
                    q8 = c * KB
                    # rows 0..7 = wh, rows 8..15 = wl; wh DMA'd only once
                    # (the hi matmul reads wq2[0:8]). Two DMA queues.
                    wq2 = wqp.tile([2 * KB, KB, OUT_CH], f16, tag="wq2")
                    nc.scalar.dma_start(
                        wq2[0:KB], wc2_d.ap()[0:KB, q8:q8 + KB, :])
                    nc.sync.dma_start(
                        wq2[KB:2 * KB], wc2_d.ap()[KB:2 * KB, q8:q8 + KB, :])

                    us = []
                    for j in range(KB):
                        q = q8 + j
                        psi = psum.tile([128, OUT_CH], f32, tag="ps")
                        for h in range(2):
                            dst = psi[:, h * 512:(h + 1) * 512]
                            nc.tensor.matmul(
                                dst, xstA[c][:, j, :],
                                wq2[0:KB, j, h * 512:(h + 1) * 512],
                                start=True, stop=False,
                                skip_group_check=True, tile_position=(0, 0))
                            nc.tensor.matmul(
                                dst, xstC[c][:, j, :],
                                wq2[:, j, h * 512:(h + 1) * 512],
                                start=False, stop=True,
                                skip_group_check=True, tile_position=(0, 0))
                        v = vpool.tile([128, OUT_CH], f16, tag="v")
                        nc.vector._custom_dve(
                            sinsq_op, out=v[:], in0=psi[:],
                            s0=MAGIC, s1=POLY_A, imm2=POLY_B)
                        u = upool.tile([128, OUT_CH], f16, tag="u")
                        _raw_act(nc, u[:], v[:], AF.Reciprocal,
                                 bias=biases[q], scale=scales[q])
                        us.append(u)
                    pairs = []
                    for jj in range(KB // 2):
                        up = ppool.tile([128, OUT_CH], f16, tag="up")
                        nc.gpsimd.tensor_add(up[:], us[2 * jj][:], us[2 * jj + 1][:])
                        pairs.append(up)
                    # lag PE accumulation by one chunk so PE never waits
                    if pend:
                        emit_acc(pend, last=False)
                    pend = pairs
                emit_acc(pend, last=True)

            if niter == 1:
                run_iter()
            else:
                with tc.For_i(0, niter, 1):
                    run_iter()

            out_sb = singles.tile([128, OUT_CH], f32)
            nc.scalar.copy(out_sb[:], out_ps[:])
            nc.sync.dma_start(out_d.ap(), out_sb[:])

    nc.compile()
    return nc


def _host_prep(weight, morr_output_scale):
    w = np.abs(np.asarray(weight, dtype=np.float32))   # [P, Q, KB]
    s = morr_output_scale - morr_output_scale.mean()
    half = s[..., :-1, :]                              # [1,1,Q//2,1]
    scale = np.concatenate([half, -half], axis=2)[0, 0, :, 0].astype(np.float32)
    sprime = (-KCONST * scale).astype(np.float64)      # folded -K

    # circulant moving-operand layout: wc[s, q, p*KB+t] = w[p, q, (t-s) % KB]
    # with 1/(2 pi) folded in so the matmul yields psi = phi/(2 pi)
    wc = np.empty((KB, Q, P * KB), np.float64)
    for sh in range(KB):
        rolled = np.roll(w, sh, axis=2)
        wc[sh] = rolled.transpose(1, 0, 2).reshape(Q, P * KB)
    wc = (wc * INV2PI).astype(np.float32)

    wh = wc.astype(np.float16)
    wl = (wc - wh.astype(np.float32)).astype(np.float16)
    wq2 = np.concatenate([wh, wl], axis=0)             # [2KB, Q, P*KB]

    # ACT affine immediates: d_q = (4 rho * v + (1-rho)^2) / s'_q
    with np.errstate(divide="ignore"):
        act_scale = 4.0 * RHO / sprime
        act_bias = MCONST / sprime
    tiny = np.abs(sprime) < 1e-30
    act_scale[tiny] = 0.0
    act_bias[tiny] = 1e30
    return (np.ascontiguousarray(wq2),
            act_scale.astype(np.float64).tolist(),
            act_bias.astype(np.float64).tolist())


def kernel(x, weight, morr_output_scale, _trace=False):
    from concourse import bass_utils

    wq2, scales, biases = _host_prep(weight, morr_output_scale)
    key = hash((tuple(scales), tuple(biases)))
    if _CACHE.get("key") != key:
        _CACHE["nc"] = _build_nc(scales, biases)
        _CACHE["key"] = key
    nc = _CACHE["nc"]

    x = np.ascontiguousarray(np.asarray(x, dtype=np.float32))
    in_maps = []
    for c in range(NCORES):
        in_maps.append({
            "x": np.ascontiguousarray(x[c * BSC:(c + 1) * BSC]),
            "wc2": wq2,
        })
    res = bass_utils.run_bass_kernel_spmd(
        nc, in_maps, core_ids=list(range(NCORES)), trace=_trace)
    out = np.concatenate([res.results[c]["out"] for c in range(NCORES)], axis=0)
    if _trace:
        _CACHE["last_results"] = res
    return out


# revision 43
# speedup vs baseline: 1.1125x; 1.0594x over previous
"""Trainium2 Bass kernel for AllPassMORRCirculantLinear.

Math (reference, per batch row b):
  xb = x.reshape(bs, q, k); xb = xb*xb
  phi[b,p,q,t] = sum_s xb[b,q,s] * |w|[p,q,(t-s) mod k]   (circular conv, k=8)
  t(phi) = (a^2 + r^2 - 2 a r cos phi) / (1 + (ar)^2 - 2 a r cos phi)
  out[b, p*k+t] = sum_q scale[q] * t(phi[b,p,q,t])

Using t = 1 - K*u, u = 1/(B - 2*rho*cos phi), sum_q scale[q] == 0:
  out = sum_q s'_q * u_q,  s'_q = -K*scale[q]
Half-angle form (pole-robust):
  B - 2 rho cos phi = (1-rho)^2 + 4 rho sin^2(phi/2)
  sin^2(phi/2) = sin^2(pi * frac(psi)),  psi = phi/(2 pi)   (sign-free!)

Per-core pipeline (data-parallel over batch, 128 rows/core):
  PE  : psi = phi/(2pi) via fp16 hi/lo matmuls (1/2pi folded into weights,
        22-bit effective mantissa) -> PSUM [128,1024] per q
  DVE : one 8-stage custom op FRAC_SINSQ: magic-number frac (exact) +
        minimax cubic s = r*(a + b r^2), v = s^2 ~ sin^2(pi r)
        (relative error weighted by pole sensitivity: 1.2e-2 max)
  ACT : u'_q = Reciprocal(scale_q * v + bias_q)  [raw InstActivation;
        scale_q = 4rho/s'_q, bias_q = (1-rho)^2/s'_q as immediates]
  Pool: pair-sums u'_{2j}+u'_{2j+1} (fp16)
  PE  : out += I @ pair  (fp16 identity-stationary accumulating matmuls
        into a dedicated PSUM bank pair; start at pair 0, stop at 63)
All ACT funcs (Square/Copy/Reciprocal/Identity) live in one table set
(reciprocal_and_small) -> single ACT_TABLE_LOAD.
"""

import sys

for _p in ("/opt/trn_rl_repo",):
    if _p not in sys.path:
        sys.path.insert(0, _p)

import numpy as np
from contextlib import ExitStack

MRR_A = 0.8682
MRR_R = 0.8602
RHO = MRR_A * MRR_R
MCONST = (1.0 - RHO) ** 2                      # (1-rho)^2
KCONST = (1.0 - MRR_A * MRR_A) * (1.0 - MRR_R * MRR_R)

BS, IN_CH, OUT_CH, KB = 1024, 1024, 1024, 8
Q = IN_CH // KB    # 128
P = OUT_CH // KB   # 128
NCORES = 8
BSC = BS // NCORES  # 128 batch rows per core
NCHUNK = 16        # q chunks of 8

MAGIC = 12582912.0  # 1.5*2^23: x + MAGIC - MAGIC == round(x) in fp32 RNE
INV2PI = 1.0 / (2.0 * np.pi)
POLY_A = 3.111744676   # minimax (pole-sensitivity weighted): sin(pi r) ~ r(A + B r^2)
POLY_B = -4.497768741

_CACHE = {}


def _sinsq_ref(in0, in1, s0, s1, imm2):
    f = np.float32
    x = in0.astype(f)
    t = (x + f(s0)).astype(f)
    k = (t - f(s0)).astype(f)
    r = (x - k).astype(f)
    r2 = (r * r).astype(f)
    g = ((r2 * f(imm2)).astype(f) + f(s1)).astype(f)
    st = (r * g).astype(f)
    return (st * st).astype(f)


def _register_frac_sinsq():
    from concourse import dve_ops
    from concourse.dve_spec import Spec, Src0, C0, C1, C2, lower
    from concourse.dve_uop import DveOpSpec

    name = "FRAC_SINSQ_ANT"
    if name in dve_ops._SUB_OPCODE_FOR_NAME:
        return next(op for op in dve_ops.OPS if op.name == name)
    t = Src0 + C0
    k = t - C0
    r = Src0 - k
    r2 = r * r
    g = r2 * C2 + C1
    st = r * g
    spec = Spec(body=st * st, reference=_sinsq_ref)
    row = max(dve_ops._SUB_OPCODE_FOR_NAME.values()) + 1
    assert row < 0x20
    dve_ops._SUB_OPCODE_FOR_NAME[name] = row
    shas = {}
    for ver in ("v3", "v4"):
        c = DveOpSpec(name=name, opcode=row, uops=lower(spec, ver=ver), rd1_en=False)
        shas[ver] = c.sha(ver)
    op = dve_ops.DveOp(name, spec, subdim=False, uops_sha=shas)
    dve_ops.OPS.append(op)
    dve_ops.CUSTOM_DVE_SPECS[name] = spec
    return op


def _raw_act(nc, out, in_, func, bias=0.0, scale=1.0):
    """Emit InstActivation directly (the bass wrapper blocks Reciprocal;
    measured accuracy here is ~1e-5 relative, ample for this kernel)."""
    from concourse import mybir
    se = nc.scalar
    inputs = [se.lower_ap(in_)]
    for arg in (bias, scale, 0.0):
        inputs.append(mybir.ImmediateValue(dtype=mybir.dt.float32, value=float(arg)))
    return se.add_instruction(
        mybir.InstActivation(
            name=nc.get_next_instruction_name(),
            func=func,
            ins=inputs,
            outs=[se.lower_ap(out)],
        )
    )


DEFAULT_OPTS = dict(
    wq_bufs=3,      # weight chunk prefetch depth
    v_bufs=6,       # DVE output tiles in flight
    u_bufs=6,       # ACT output tiles in flight
    p_bufs=28,      # pool pair-sum tiles in flight
    acc_lag=6,      # chunks of delay before PE consumes pair tiles
    use_pairs=True,  # pool pre-sums pairs; False = PE accumulates all 128
    interleave_acc=True,  # spread acc matmuls between psi matmuls
    act_copy_psum=False,  # ACT copies psi PSUM->SBUF; DVE reads SBUF
    recip_lag=2,          # q-lag between ACT copy and ACT recip emission
    half_cols=False,      # 512-col units: psi = 1 PSUM bank, 6-deep
    psum_bufs=None,       # override psi pool depth
    wsplit=8,             # q's per weight DMA tile (8 = whole chunk)
    # ablation switches (timing experiments only; break numerics)
    skip_recip=False, skip_dve=False, preload_weights=False, skip_acc=False,
)


def _build_nc(scales, biases, niter=1, opts=None):
    from concourse import bacc, mybir
    import concourse.tile as tile
    from concourse import masks

    o = dict(DEFAULT_OPTS)
    if opts:
        o.update(opts)

    sinsq_op = _register_frac_sinsq()

    nc = bacc.Bacc("TRN2", debug=False)
    f32 = mybir.dt.float32
    f16 = mybir.dt.float16
    AF = mybir.ActivationFunctionType

    x_d = nc.dram_tensor("x", [BSC, IN_CH], f32, kind="ExternalInput")
    wc2_d = nc.dram_tensor("wc2", [2 * KB, Q, OUT_CH], f16, kind="ExternalInput")
    out_d = nc.dram_tensor("out", [BSC, OUT_CH], f32, kind="ExternalOutput")

    with tile.TileContext(nc) as tc:
        with ExitStack() as ctx:
            singles = ctx.enter_context(tc.tile_pool(name="singles", bufs=1))
            wqp = ctx.enter_context(tc.tile_pool(name="wqp", bufs=o["wq_bufs"]))
            vpool = ctx.enter_context(tc.tile_pool(name="vpool", bufs=o["v_bufs"]))
            vcpool = (ctx.enter_context(tc.tile_pool(name="vcpool", bufs=4))
                      if o["act_copy_psum"] else None)
            upool = ctx.enter_context(tc.tile_pool(name="upool", bufs=o["u_bufs"]))
            ppool = ctx.enter_context(tc.tile_pool(name="ppool", bufs=o["p_bufs"]))

            ident = singles.tile([128, 128], f16)
            masks.make_identity(nc, ident[:])

            x_sb = singles.tile([128, IN_CH], f32)
            nc.sync.dma_start(x_sb[:], x_d.ap())
            # input intensity modulation: x <- x^2 (in place)
            nc.scalar.activation(x_sb[:], x_sb[:], AF.Square)

            # interleaved hi/lo staging tile: [part, q, s-slot(16)]
            # slots 0..7 = residual (xl), slots 8..15 = fp16(xq) (xh)
            xhl = singles.tile([128, Q, 16], f16)
            xq3 = x_sb[:].rearrange("p (q s) -> p q s", s=KB)
            nc.scalar.copy(xhl[:, :, KB:2 * KB], xq3)
            nc.vector.tensor_sub(xhl[:, :, 0:KB], xq3, xhl[:, :, KB:2 * KB])

            # transpose to xstC[g] = [xl(0-7); xh(8-15)] x [j(8), b(128)];
            # xstA[g] = xh alone at partitions 0-7 (SBUF->SBUF DMA dup)
            xstC, xstA = [], []
            with tc.tile_pool(name="tps", bufs=2, space="PSUM") as tpp:
                for g in range(NCHUNK):
                    tp = tpp.tile([16, KB * 128], f16, tag="tp")
                    for j in range(KB):
                        nc.tensor.transpose(
                            tp[:, j * 128:(j + 1) * 128],
                            xhl[:, g * KB + j, :], ident[:])
                    xc = singles.tile([16, KB, 128], f16, tag=f"xstC{g}")
                    nc.scalar.copy(xc[:].rearrange("s j b -> s (j b)"), tp[:])
                    xstC.append(xc)
                    xa = singles.tile([8, KB, 128], f16, tag=f"xstA{g}")
                    nc.sync.dma_start(xa[:], xc[KB:2 * KB])
                    xstA.append(xa)

            psum_out = ctx.enter_context(
                tc.tile_pool(name="psout", bufs=1, space="PSUM"))
            n_psum = o["psum_bufs"] or (6 if o["half_cols"] else 3)
            psum = ctx.enter_context(
                tc.tile_pool(name="psum", bufs=n_psum, space="PSUM"))

            if o["half_cols"]:
                out_ps2 = []
                for h in range(2):
                    oph = psum_out.tile([128, 512], f32, tag=f"o{h}")
                    out_ps2.append(oph)
            else:
                out_ps = psum_out.tile([128, OUT_CH], f32)

            vstatic = None
            if o["skip_dve"]:
                vstatic = singles.tile([128, OUT_CH], f16)
                nc.vector.memset(vstatic[:], 0.5)
            wq_static = None
            if o["preload_weights"]:
                # one shared tile (timing ablation only — wrong numerics)
                wt = singles.tile([2 * KB, KB, OUT_CH], f16, tag="wqs")
                nc.scalar.dma_start(wt[0:KB], wc2_d.ap()[0:KB, 0:KB, :])
                nc.sync.dma_start(wt[KB:2 * KB], wc2_d.ap()[KB:2 * KB, 0:KB, :])
                wq_static = [wt] * NCHUNK

            def run_iter_half():
                pend = []   # (pair tile, half) awaiting PE accumulation
                rq = []     # (v, q, half) awaiting lagged ACT recip
                uq = [[], []]
                nacc = [0, 0]
                n_per_half = Q // 2 if o["use_pairs"] else Q

                def emit_acc_h(up, h):
                    nc.tensor.matmul(
                        out_ps2[h][:], ident[:], up[:],
                        start=(nacc[h] == 0),
                        stop=(nacc[h] == n_per_half - 1),
                        skip_group_check=True, tile_position=(0, 0))
                    nacc[h] += 1

                unit = [0]
                for c in range(NCHUNK):
                    q8 = c * KB
                    wq2 = wqp.tile([2 * KB, KB, OUT_CH], f16, tag="wq2")
                    nc.scalar.dma_start(
                        wq2[0:KB], wc2_d.ap()[0:KB, q8:q8 + KB, :])
                    nc.sync.dma_start(
                        wq2[KB:2 * KB], wc2_d.ap()[KB:2 * KB, q8:q8 + KB, :])
                    for j in range(KB):
                        q = q8 + j
                        for h in range(2):
                            psi = psum.tile([128, 512], f32, tag="psh")
                            nc.tensor.matmul(
                                psi[:], xstA[c][:, j, :],
                                wq2[0:KB, j, h * 512:(h + 1) * 512],
                                start=True, stop=False,
                                skip_group_check=True, tile_position=(0, 0))
                            nc.tensor.matmul(
                                psi[:], xstC[c][:, j, :],
                                wq2[:, j, h * 512:(h + 1) * 512],
                                start=False, stop=True,
                                skip_group_check=True, tile_position=(0, 0))
                            if (pend and c >= o["acc_lag"]
                                    and unit[0] % (2 if o["use_pairs"] else 1) == 0):
                                ut, uh = pend.pop(0)
                                emit_acc_h(ut, uh)
                            unit[0] += 1
                            v = vpool.tile([128, 512], f16, tag="vh")
                            nc.vector._custom_dve(
                                sinsq_op, out=v[:], in0=psi[:],
                                s0=MAGIC, s1=POLY_A, imm2=POLY_B)
                            rq.append((v, q, h))
                            if len(rq) > 2 * o["recip_lag"]:
                                vv, qq, hh = rq.pop(0)
                                u = upool.tile([128, 512], f16, tag="uh")
                                _raw_act(nc, u[:], vv[:], AF.Reciprocal,
                                         bias=biases[qq], scale=scales[qq])
                                uq[hh].append(u)
                                if o["use_pairs"]:
                                    if len(uq[hh]) >= 2:
                                        up = ppool.tile([128, 512], f16,
                                                        tag="uph")
                                        nc.gpsimd.tensor_add(
                                            up[:], uq[hh].pop(0)[:],
                                            uq[hh].pop(0)[:])
                                        pend.append((up, hh))
                                else:
                                    pend.append((uq[hh].pop(0), hh))
                while rq:
                    vv, qq, hh = rq.pop(0)
                    u = upool.tile([128, 512], f16, tag="uh")
                    _raw_act(nc, u[:], vv[:], AF.Reciprocal,
                             bias=biases[qq], scale=scales[qq])
                    uq[hh].append(u)
                for h in range(2):
                    if o["use_pairs"]:
                        while len(uq[h]) >= 2:
                            up = ppool.tile([128, 512], f16, tag="uph")
                            nc.gpsimd.tensor_add(
                                up[:], uq[h].pop(0)[:], uq[h].pop(0)[:])
                            pend.append((up, h))
                    pend.extend((u, h) for u in uq[h])
                    uq[h] = []
                for ut, uh in pend:
                    emit_acc_h(ut, uh)

            def run_iter():
                if o["half_cols"]:
                    return run_iter_half()
                pend = []  # acc-input tiles awaiting PE accumulation
                rq = []    # (v tile, q) awaiting the lagged ACT recip
                uq = []    # u tiles awaiting pair-sum / acc
                nacc = [0]
                n_acc_total = NCHUNK * (KB // 2 if o["use_pairs"] else KB)
                if not o["use_pairs"]:
                    n_acc_total = Q

                def emit_one_acc(up):
                    for h in range(2):
                        nc.tensor.matmul(
                            out_ps[:, h * 512:(h + 1) * 512],
                            ident[:], up[:, h * 512:(h + 1) * 512],
                            start=(nacc[0] == 0),
                            stop=(nacc[0] == n_acc_total - 1),
                            skip_group_check=True, tile_position=(0, 0))
                    nacc[0] += 1

                ws = o["wsplit"]
                for c in range(NCHUNK):
                    q8 = c * KB
                    # rows 0..7 = wh, rows 8..15 = wl; wh DMA'd only once
                    # (the hi matmul reads wq2[0:8]). Two DMA queues,
                    # ws q's per sub-tile for finer arrival granularity.
                    if o["preload_weights"]:
                        wsub = [wq_static[c]]
                    else:
                        wsub = []
                        for s in range(KB // ws):
                            qa = q8 + s * ws
                            wt = wqp.tile([2 * KB, ws, OUT_CH], f16, tag="wq2")
                            nc.scalar.dma_start(
                                wt[0:KB], wc2_d.ap()[0:KB, qa:qa + ws, :])
                            nc.sync.dma_start(
                                wt[KB:2 * KB],
                                wc2_d.ap()[KB:2 * KB, qa:qa + ws, :])
                            wsub.append(wt)

                    us = []
                    for j in range(KB):
                        q = q8 + j
                        wq2 = wsub[j // ws if not o["preload_weights"] else 0]
                        jw = j % ws if not o["preload_weights"] else j
                        psi = psum.tile([128, OUT_CH], f32, tag="ps")
                        for h in range(2):
                            dst = psi[:, h * 512:(h + 1) * 512]
                            nc.tensor.matmul(
                                dst, xstA[c][:, j, :],
                                wq2[0:KB, jw, h * 512:(h + 1) * 512],
                                start=True, stop=False,
                                skip_group_check=True, tile_position=(0, 0))
                            nc.tensor.matmul(
                                dst, xstC[c][:, j, :],
                                wq2[:, jw, h * 512:(h + 1) * 512],
                                start=False, stop=True,
                                skip_group_check=True, tile_position=(0, 0))
                        # interleave lagged acc matmuls at the production rate
                        # (pairs: 4/chunk on even j; singles: 8/chunk)
                        if (o["interleave_acc"] and pend and c >= o["acc_lag"]
                                and (not o["use_pairs"] or j % 2 == 0)):
                            emit_one_acc(pend.pop(0))
                        if o["skip_dve"]:
                            v = vpool.tile([128, 64], f16, tag="vt")
                            nc.vector._custom_dve(
                                sinsq_op, out=v[:], in0=psi[:, 0:64],
                                s0=MAGIC, s1=POLY_A, imm2=POLY_B)
                            v = vstatic
                        elif o["act_copy_psum"]:
                            vc = vcpool.tile([128, OUT_CH], f32, tag="vc")
                            nc.scalar.copy(vc[:], psi[:])
                            v = vpool.tile([128, OUT_CH], f16, tag="v")
                            nc.vector._custom_dve(
                                sinsq_op, out=v[:], in0=vc[:],
                                s0=MAGIC, s1=POLY_A, imm2=POLY_B)
                        else:
                            v = vpool.tile([128, OUT_CH], f16, tag="v")
                            nc.vector._custom_dve(
                                sinsq_op, out=v[:], in0=psi[:],
                                s0=MAGIC, s1=POLY_A, imm2=POLY_B)
                        if o["skip_recip"]:
                            uq.append(v)
                        else:
                            rq.append((v, q))
                            if len(rq) > o["recip_lag"]:
                                vv, qq = rq.pop(0)
                                u = upool.tile([128, OUT_CH], f16, tag="u")
                                _raw_act(nc, u[:], vv[:], AF.Reciprocal,
                                         bias=biases[qq], scale=scales[qq])
                                uq.append(u)
                    if o["skip_acc"]:
                        while uq:
                            uu = uq.pop(0)
                            vt = vpool.tile([128, 64], f16, tag="vt2")
                            nc.vector._custom_dve(
                                sinsq_op, out=vt[:], in0=uu[:, 0:64],
                                s0=MAGIC, s1=POLY_A, imm2=POLY_B)
                        continue
                    if o["use_pairs"]:
                        while len(uq) >= 2:
                            up = ppool.tile([128, OUT_CH], f16, tag="up")
                            nc.gpsimd.tensor_add(up[:], uq.pop(0)[:], uq.pop(0)[:])
                            pend.append(up)
                    else:
                        pend.extend(uq)
                        uq = []
                    if not o["interleave_acc"]:
                        keep = 4 * o["acc_lag"]
                        while len(pend) > keep:
                            emit_one_acc(pend.pop(0))
                # drain: finish lagged recips, final pairs, then acc
                if not o["skip_acc"]:
                    while rq:
                        vv, qq = rq.pop(0)
                        u = upool.tile([128, OUT_CH], f16, tag="u")
                        _raw_act(nc, u[:], vv[:], AF.Reciprocal,
                                 bias=biases[qq], scale=scales[qq])
                        uq.append(u)
                    if o["use_pairs"]:
                        while len(uq) >= 2:
                            up = ppool.tile([128, OUT_CH], f16, tag="up")
                            nc.gpsimd.tensor_add(up[:], uq.pop(0)[:], uq.pop(0)[:])
                            pend.append(up)
                    pend.extend(uq)
                    for up in pend:
                        emit_one_acc(up)

            if niter == 1:
                run_iter()
            else:
                with tc.For_i(0, niter, 1):
                    run_iter()

            out_sb = singles.tile([128, OUT_CH], f32)
            if o["half_cols"]:
                nc.scalar.copy(out_sb[:, 0:512], out_ps2[0][:])
                nc.scalar.copy(out_sb[:, 512:1024], out_ps2[1][:])
            else:
                nc.scalar.copy(out_sb[:], out_ps[:])
            nc.sync.dma_start(out_d.ap(), out_sb[:])

    nc.compile()
    return nc


def _host_prep(weight, morr_output_scale):
    w = np.abs(np.asarray(weight, dtype=np.float32))   # [P, Q, KB]
    s = morr_output_scale - morr_output_scale.mean()
    half = s[..., :-1, :]                              # [1,1,Q//2,1]
    scale = np.concatenate([half, -half], axis=2)[0, 0, :, 0].astype(np.float32)
    sprime = (-KCONST * scale).astype(np.float64)      # folded -K

    # circulant moving-operand layout: wc[s, q, p*KB+t] = w[p, q, (t-s) % KB]
    # with 1/(2 pi) folded in so the matmul yields psi = phi/(2 pi)
    wc = np.empty((KB, Q, P * KB), np.float64)
    for sh in range(KB):
        rolled = np.roll(w, sh, axis=2)
        wc[sh] = rolled.transpose(1, 0, 2).reshape(Q, P * KB)
    wc = (wc * INV2PI).astype(np.float32)

    wh = wc.astype(np.float16)
    wl = (wc - wh.astype(np.float32)).astype(np.float16)
    wq2 = np.concatenate([wh, wl], axis=0)             # [2KB, Q, P*KB]

    # ACT affine immediates: d_q = (4 rho * v + (1-rho)^2) / s'_q
    with np.errstate(divide="ignore"):
        act_scale = 4.0 * RHO / sprime
        act_bias = MCONST / sprime
    tiny = np.abs(sprime) < 1e-30
    act_scale[tiny] = 0.0
    act_bias[tiny] = 1e30
    return (np.ascontiguousarray(wq2),
            act_scale.astype(np.float64).tolist(),
            act_bias.astype(np.float64).tolist())


def kernel(x, weight, morr_output_scale, _trace=False):
    from concourse import bass_utils

    wq2, scales, biases = _host_prep(weight, morr_output_scale)
    key = hash((tuple(scales), tuple(biases)))
    if _CACHE.get("key") != key:
        _CACHE["nc"] = _build_nc(scales, biases)
        _CACHE["key"] = key
    nc = _CACHE["nc"]

    x = np.ascontiguousarray(np.asarray(x, dtype=np.float32))
    in_maps = []
    for c in range(NCORES):
        in_maps.append({
            "x": np.ascontiguousarray(x[c * BSC:(c + 1) * BSC]),
            "wc2": wq2,
        })
    res = bass_utils.run_bass_kernel_spmd(
        nc, in_maps, core_ids=list(range(NCORES)), trace=_trace)
    out = np.concatenate([res.results[c]["out"] for c in range(NCORES)], axis=0)
    if _trace:
        _CACHE["last_results"] = res
    return out
